# revision 1
# baseline (speedup 1.0000x reference)
"""V3: routed kernel via operand-swapped matmuls + native permute chain.

Data-parallel over 8 cores (1024 rows each), weights replicated (bf16 for the
big per-expert stacks).  Host sorts each core's rows by expert per module
type; groups are padded to C=288 (static program; C bumps only if some group
ever exceeds it).  Routed big layers run 4x fewer FLOPs than dense:

- L0_1/L1_1/L2_1 run operand-swapped (lhsT = activation column block, rhs =
  that group's expert weights), so PSUM output is batch-major; ReLU
  evacuation writes a chunk-major token tile which a native indirect-scatter
  DMA writes to DRAM rows in the NEXT stage's expert order, and XBAR
  transpose-DMAs load it back feature-major.  The inter-stage all-to-all
  costs two 1.2 MB DMA hops and no custom GPSIMD ucode.
- L3_1 and the head run feature-major grouped (no transition after them).
- First layers use the order-agnostic expanded-input trick (fp32r, K=128).
- Biases are exact fp32: ACT bias operand (feature-major layers) or a K=1
  ones-outer-product matmul into PSUM (batch-major layers).
"""

import numpy as np
import ml_dtypes
from contextlib import ExitStack

import concourse.bass as bass
import concourse.bacc as bacc
import concourse.tile as tile
import concourse.mybir as mybir
from concourse import bass_utils

F32 = mybir.dt.float32
F32R = mybir.dt.float32r
BF16 = mybir.dt.bfloat16
I32 = mybir.dt.int32
RELU = mybir.ActivationFunctionType.Relu
COPY = mybir.ActivationFunctionType.Copy

B = 8192
NCORES = 8
BC = B // NCORES
FEAT = 32
M = 4
H = 512
OUT = 8
P = 128
C0 = 288                     # default group capacity (multiple of 32)
KBIG = [4, 8, 8, 8]


def _chunks(C):
    """Static chunk split of each group: [(m, g0, r)] with r<=128."""
    out = []
    for m in range(M):
        off = 0
        while off < C:
            r = min(P, C - off)
            out.append((m, C * m + off, r))
            off += r
    return out


def _emit(nc, tc, ctx, d, C):
    Bp = M * C
    chunks = _chunks(C)
    NCH = len(chunks)

    consts = ctx.enter_context(tc.tile_pool(name="consts", bufs=1))
    wpool = ctx.enter_context(tc.tile_pool(name="wbig", bufs=16))
    hp = ctx.enter_context(tc.tile_pool(name="hacts", bufs=1))
    permp = ctx.enter_context(tc.tile_pool(name="perm", bufs=1))
    outp = ctx.enter_context(tc.tile_pool(name="outs", bufs=1))
    psp = ctx.enter_context(tc.tile_pool(name="psum", bufs=8, space="PSUM"))

    # ---------------- constants ----------------
    xe_t = []
    for j in range(4):
        t = consts.tile([P, Bp], F32R, tag=f"xe{j}", name=f"xe{j}")
        nc.sync.dma_start(t[:], d["xe"].ap()[j, :, :])
        xe_t.append(t)
    wf_t = []
    for j in range(4):
        t = consts.tile([P, H], F32R, tag=f"wf{j}", name=f"wf{j}")
        nc.sync.dma_start(t[:], d["Wf"].ap()[j, :, :])
        wf_t.append(t)
    w32_t = []
    for m in range(M):
        t = consts.tile([P, 4, OUT], BF16, tag=f"w32_{m}", name=f"w32_{m}")
        nc.sync.dma_start(
            t[:], d["W32"].ap()[m, :, :].rearrange("(a p) o -> p a o", p=P))
        w32_t.append(t)
    bias_sb = consts.tile([P, 8 * 16], F32, tag="bias", name="bias")
    nc.sync.dma_start(bias_sb[:], d["bias"].ap())
    bh = consts.tile([OUT, 4], F32, tag="bh", name="bh")
    nc.sync.dma_start(bh[:], d["bh"].ap())
    brow = consts.tile([1, 3 * 4 * H], F32R, tag="brow", name="brow")
    nc.sync.dma_start(brow[:], d["brow"].ap())
    ones = consts.tile([1, P], F32R, tag="ones", name="ones")
    nc.sync.dma_start(ones[:], d["ones"].ap())
    tbl = consts.tile([P, 3 * NCH], I32, tag="tbl", name="tbl")
    nc.sync.dma_start(tbl[:], d["tbl"].ap())

    def bias_ap(layer, hh, m):
        col = layer * 16 + hh * 4 + m
        return bias_sb[:, col:col + 1]

    # ---------------- layers ----------------
    def first_layer(j, tag):
        """relu(Wf[j].T @ xe_g[j] + b_j0): 4x [128, Bp] bf16, feature-major."""
        outs = []
        for hpair in range(2):
            ps = [[psp.tile([P, C], F32, tag="pt", name="pt")
                   for m in range(M)] for _ in range(2)]
            for hi in range(2):
                hh = hpair * 2 + hi
                for m in range(M):
                    nc.tensor.matmul(
                        ps[hi][m][:], wf_t[j][:, bass.ts(hh, P)],
                        xe_t[j][:, bass.ts(m, C)], start=True, stop=True)
            for hi in range(2):
                hh = hpair * 2 + hi
                t = hp.tile([P, Bp], BF16, tag=f"{tag}{hh}", name=f"{tag}{hh}")
                for m in range(M):
                    nc.scalar.activation(t[:, bass.ts(m, C)], ps[hi][m][:],
                                         RELU, bias=bias_ap(2 * j, hh, m))
                outs.append(t)
        return outs

    def load_w(j, m, Kc):
        ws = []
        for k in range(Kc):
            w = wpool.tile([P, H], BF16, tag="wt", name="wt")
            nc.sync.dma_start(w[:], d[f"W{j}1"].ap()[m, k * P:(k + 1) * P, :])
            ws.append(w)
        return ws

    def swapped_big(j, z_tiles):
        """relu(W_j1[expert].T @ z + b), batch-major out -> xsc token tile."""
        Kc = KBIG[j]
        xsc = permp.tile([P, NCH, H], BF16, tag="xsc", name="xsc", bufs=2)
        for m in range(M):
            ws = load_w(j, m, Kc)
            for ch, (mm, g0, r) in enumerate(chunks):
                if mm != m:
                    continue
                pb = psp.tile([P, H], F32, tag="pt", name="pt")
                nc.tensor.matmul(pb[:r, :], ones[:, :r],
                                 brow[:, (j * 4 + m) * H:(j * 4 + m + 1) * H],
                                 start=True, stop=False)
                for k in range(Kc):
                    nc.tensor.matmul(pb[:r, :], z_tiles[k][:, g0:g0 + r],
                                     ws[k][:],
                                     start=False, stop=(k == Kc - 1))
                nc.scalar.activation(xsc[:r, ch, :], pb[:r, :], RELU)
        return xsc

    def transition(t_i, xsc):
        """Scatter chunk tokens into next stage's order; XBAR back."""
        xb = d["xb"][t_i]
        for ch, (m, g0, r) in enumerate(chunks):
            nc.gpsimd.indirect_dma_start(
                xb.ap(),
                bass.IndirectOffsetOnAxis(
                    ap=tbl[:r, t_i * NCH + ch:t_i * NCH + ch + 1], axis=0),
                xsc[:r, ch, :], None)
        zx = permp.tile([P, 4, Bp], BF16, tag="zx", name="zx")
        for k in range(4):
            nc.sync.dma_start(zx[:, k, :], xb.ap()[:, k * P:(k + 1) * P],
                              transpose=True)
        return [zx[:, k, :] for k in range(4)]

    def grouped_big(j, z_tiles, tag):
        """relu(W_j1[expert].T @ z + b): feature-major grouped output."""
        Kc = KBIG[j]
        outs = [hp.tile([P, Bp], BF16, tag=f"{tag}{hh}", name=f"{tag}{hh}")
                for hh in range(4)]
        for m in range(M):
            ws = load_w(j, m, Kc)
            ps = [psp.tile([P, C], F32, tag="pt", name="pt")
                  for hh in range(4)]
            for k in range(Kc):
                for hh in range(4):
                    nc.tensor.matmul(
                        ps[hh][:], ws[k][:, bass.ts(hh, P)],
                        z_tiles[k][:, bass.ts(m, C)],
                        start=(k == 0), stop=(k == Kc - 1))
            for hh in range(4):
                nc.scalar.activation(outs[hh][:, bass.ts(m, C)], ps[hh][:],
                                     RELU, bias=bias_ap(2 * j + 1, hh, m))
        return outs

    # ---------------- network ----------------
    x = first_layer(0, "h")
    xsc = swapped_big(0, x)
    zx = transition(0, xsc)
    h1 = first_layer(1, "g")
    xsc = swapped_big(1, zx + h1)
    zx = transition(1, xsc)
    h2 = first_layer(2, "h")
    xsc = swapped_big(2, zx + h2)
    zx = transition(2, xsc)
    h3 = first_layer(3, "g")
    x4 = grouped_big(3, zx + h3, "x4")

    # head
    ps = [psp.tile([OUT, C], F32, tag="pt", name="pt") for m in range(M)]
    for k in range(4):
        for m in range(M):
            nc.tensor.matmul(ps[m][:], w32_t[m][:, k, :],
                             x4[k][:, bass.ts(m, C)],
                             start=(k == 0), stop=(k == 3))
    out_t = outp.tile([OUT, Bp], F32, tag="outt", name="outt")
    for m in range(M):
        nc.scalar.activation(out_t[:, bass.ts(m, C)], ps[m][:], COPY)
        nc.vector.tensor_scalar_add(out_t[:, bass.ts(m, C)],
                                    out_t[:, bass.ts(m, C)], bh[:, m:m + 1])
    nc.sync.dma_start(d["out"].ap(), out_t[:])


def build_program(C=C0, reps: int = 1):
    Bp = M * C
    NCH = len(_chunks(C))
    nc = bacc.Bacc("TRN2", target_bir_lowering=False, debug=False,
                   enable_asserts=False)
    d = {}
    d["xe"] = nc.dram_tensor("xe", [4, P, Bp], F32R, kind="ExternalInput")
    d["Wf"] = nc.dram_tensor("Wf", [4, P, H], F32R, kind="ExternalInput")
    d["W01"] = nc.dram_tensor("W01", [M, H, H], BF16, kind="ExternalInput")
    d["W11"] = nc.dram_tensor("W11", [M, 2 * H, H], BF16, kind="ExternalInput")
    d["W21"] = nc.dram_tensor("W21", [M, 2 * H, H], BF16, kind="ExternalInput")
    d["W31"] = nc.dram_tensor("W31", [M, 2 * H, H], BF16, kind="ExternalInput")
    d["W32"] = nc.dram_tensor("W32", [M, H, OUT], BF16, kind="ExternalInput")
    d["bias"] = nc.dram_tensor("bias", [P, 8 * 16], F32, kind="ExternalInput")
    d["bh"] = nc.dram_tensor("bh", [OUT, 4], F32, kind="ExternalInput")
    d["brow"] = nc.dram_tensor("brow", [1, 3 * 4 * H], F32R,
                               kind="ExternalInput")
    d["ones"] = nc.dram_tensor("ones", [1, P], F32R, kind="ExternalInput")
    d["tbl"] = nc.dram_tensor("tbl", [P, 3 * NCH], I32, kind="ExternalInput")
    d["out"] = nc.dram_tensor("out", [OUT, Bp], F32, kind="ExternalOutput")
    d["xb"] = [nc.dram_tensor(f"xb{i}", [Bp, H], BF16, kind="Internal")
               for i in range(3)]

    with tile.TileContext(nc) as tc, ExitStack() as ctx:
        if reps == 1:
            _emit(nc, tc, ctx, d, C)
        else:
            with tc.For_i(0, reps, 1):
                _emit(nc, tc, ctx, d, C)
    nc.compile()
    return nc


def prep_inputs(inputs):
    iv = np.asarray(inputs["input_val"], dtype=np.float32)
    feats = iv[:, :4 * FEAT]
    oh = iv[:, 4 * FEAT:4 * FEAT + 16]
    idx = [np.argmax(oh[:, 4 * j:4 * j + 4], axis=1) for j in range(4)]

    Cmax = 0
    for c in range(NCORES):
        rc = slice(c * BC, (c + 1) * BC)
        for j in range(4):
            Cmax = max(Cmax, int(np.bincount(idx[j][rc], minlength=M).max()))
    C = max(C0, ((Cmax + 31) // 32) * 32)
    Bp = M * C
    chunks = _chunks(C)
    NCH = len(chunks)

    bias = np.zeros((P, 8 * 16), np.float32)
    for j in range(4):
        bl = np.asarray(inputs[f"b{j}_0"], np.float32)
        for hh in range(4):
            for m in range(M):
                bias[:, 2 * j * 16 + hh * 4 + m] = bl[m, hh * P:(hh + 1) * P]
    b31 = np.asarray(inputs["b3_1"], np.float32)
    for hh in range(4):
        for m in range(M):
            bias[:, 7 * 16 + hh * 4 + m] = b31[m, hh * P:(hh + 1) * P]
    brow = np.zeros((1, 3 * 4 * H), np.float32)
    for t, nm in enumerate(("b0_1", "b1_1", "b2_1")):
        bl = np.asarray(inputs[nm], np.float32)
        for m in range(M):
            brow[0, (t * 4 + m) * H:(t * 4 + m + 1) * H] = bl[m]
    bh = np.ascontiguousarray(np.asarray(inputs["b3_2"], np.float32).T)
    ones = np.ones((1, P), np.float32)

    Wf = np.stack([np.asarray(inputs[f"W{j}_0"], np.float32).reshape(P, H)
                   for j in range(4)])
    tobf = lambda a: np.ascontiguousarray(
        np.asarray(a, np.float32).astype(ml_dtypes.bfloat16))
    shared = {
        "Wf": np.ascontiguousarray(Wf), "bias": bias, "bh": bh,
        "brow": brow, "ones": ones,
        "W01": tobf(inputs["W0_1"]), "W11": tobf(inputs["W1_1"]),
        "W21": tobf(inputs["W2_1"]), "W31": tobf(inputs["W3_1"]),
        "W32": tobf(inputs["W3_2"]),
    }

    in_maps, meta = [], []
    for c in range(NCORES):
        rc = slice(c * BC, (c + 1) * BC)
        orders, slots, padlists = [], [], []
        for j in range(4):
            ij = idx[j][rc]
            order = np.full(Bp, -1, np.int64)
            slot = np.empty(BC, np.int64)
            pads = []
            for m in range(M):
                rows = np.nonzero(ij == m)[0]
                order[C * m:C * m + len(rows)] = rows
                slot[rows] = C * m + np.arange(len(rows))
                pads.extend(range(C * m + len(rows), C * (m + 1)))
            orders.append(order)
            slots.append(slot)
            padlists.append(np.array(pads, np.int64))

        xe = np.zeros((4, P, Bp), np.float32)
        for j in range(4):
            ij = idx[j][rc]
            fj = feats[rc, FEAT * j:FEAT * (j + 1)]
            for m in range(M):
                rows = np.nonzero(ij == m)[0]
                xe[j, m * FEAT:(m + 1) * FEAT, C * m:C * m + len(rows)] = \
                    fj[rows].T

        tblv = np.full((P, 3 * NCH), Bp + 7, np.int32)   # default: OOB skip
        for t in range(3):
            jp, jn = t, t + 1
            padmap = {int(g): i for i, g in enumerate(padlists[jp])}
            for ch, (m, g0, r) in enumerate(chunks):
                for p in range(r):
                    g = g0 + p
                    s = orders[jp][g]
                    if s >= 0:
                        tblv[p, t * NCH + ch] = slots[jn][s]
                    else:
                        tblv[p, t * NCH + ch] = padlists[jn][padmap[g]]
        in_maps.append({"xe": xe, "tbl": tblv, **shared})
        meta.append(slots[3])
    return C, in_maps, meta


_CACHE = {}


def kernel(**inputs):
    C, in_maps, meta = prep_inputs(inputs)
    if ("nc", C) not in _CACHE:
        _CACHE[("nc", C)] = build_program(C)
    nc = _CACHE[("nc", C)]
    res = bass_utils.run_bass_kernel_spmd(
        nc, in_maps, core_ids=list(range(NCORES)))
    out = np.empty((B, OUT), np.float32)
    for c in range(NCORES):
        o = res.results[c]["out"]
        out[c * BC:(c + 1) * BC] = o[:, meta[c]].T
    return out


if __name__ == "__main__":
    import sys, jax
    import reference
    cpu = jax.local_devices(backend="cpu")[0]
    with jax.default_device(cpu):
        inputs = {k: np.asarray(v) for k, v in reference.setup_inputs().items()}
        exp = np.asarray(reference.reference(**inputs))
    if len(sys.argv) > 1 and sys.argv[1] == "sim":
        from concourse.bass_interp import CoreSim
        C, in_maps, meta = prep_inputs(inputs)
        nc = build_program(C)
        sim = CoreSim(nc, trace=False)
        for k, v in in_maps[0].items():
            sim.tensor(k)[:] = v
        sim.simulate()
        o = np.asarray(sim.tensor("out"))
        got0 = o[:, meta[0]].T
        exp0 = exp[:BC]
        err = np.abs(got0 - exp0)
        print(f"sim core0 max abs err: {err.max():.3e}  "
              f"rel: {err.max()/np.abs(exp0).max():.3e}")
    else:
        got = kernel(**inputs)
        err = np.abs(got - exp)
        print(f"max abs err: {err.max():.3e}   "
              f"rel: {err.max()/np.abs(exp).max():.3e}")

